# revision 6
# baseline (speedup 1.0000x reference)
"""CoAttention kernel for Trainium2 (Bass/Tile), 8-core data-parallel.

Reference computation (per batch b):
    v2 = v @ Wv.T + bv                  [Lv, D]
    q2 = q @ Wq.T + bq                  [Lq, D]
    h  = tanh(v2[None,:,:] + q2[:,None,:])   [Lq, Lv, D]
    sv = h @ Whv ; sq = h @ Whq         [Lq, Lv]   (+bhv/+bhq: shift-invariant
                                                    under softmax -> dropped)
    av = softmax(sv, axis=v) ; aq = softmax(sq, axis=q)
    v_att = av @ v2                     [Lq, D]
    q_att = aq.T @ q2                   [Lv, D]

Sharding: pure data parallel over batch (16 batches / 8 cores = 2 per core).

Per-core dataflow (all shapes fp32):
  - load v [2,196,512], q [2,32,512]; weights pre-transposed on host:
    WvT/WqT [d,e], Wh2 [e,2] = stack(Whv,Whq), bv/bq [512,1], eye [128,128]
  - PE-transpose v,q -> vT,qT (d on partitions)
  - v2T[e,v] / q2T[e,q] via matmul, bias added during PSUM eviction
  - per e-tile (4 of 128): S[e, q*196+v] = v2T[e,v] + q2T[e,q] via ONE
    broadcast tensor_tensor add (DVE or GPSIMD, balanced), then one big
    in-place tanh on ACT ([128, 6272] per instruction)
  - scores, transposed orientation: scT[v, 2q+j] = sum_e S[e,qv] * Wh2[e,j]
    PE matmuls with S slices stationary, accumulated across e-tiles in 2
    PSUM banks (single start/stop per bank, interleaved groups)
  - softmax: sv rows via PE transpose (reduce over free v), sq columns
    directly (reduce over free q); normalization folded into the output
    matmul PSUM eviction as a per-partition ACT scale
  - v_att = exp(sv)@v2 * rv ; q_att = exp(sq).T@q2 * rq ; DMA out
"""

import numpy as np

import concourse.bass as bass
import concourse.mybir as mybir
import concourse.tile as tile
from concourse import bacc
from concourse.bass_utils import run_bass_kernel_spmd

F32 = mybir.dt.float32
AF = mybir.ActivationFunctionType
ALU = mybir.AluOpType

B, LV, LQ, D = 16, 196, 32, 512
NCORES = 8
BL = B // NCORES          # batches per core
NT = D // 128             # 4 partition tiles of the feature dims
LV0, LV1 = 128, LV - 128  # 196 = 128 + 68
QV = LQ * LV              # 6272 = flattened (q, v) free dim

# (batch, etile) units whose broadcast-add runs on GPSIMD instead of DVE
# (DVE ~6.7us/unit, GPSIMD ~13.8us/unit; 3/8 on GPSIMD balances both
#  under the ~44us ACT tanh floor)
_GPSIMD_UNITS = {(0, 1), (1, 0), (1, 2)}

_NC_CACHE = None


def _build_program():
    nc = bacc.Bacc(
        "TRN2", target_bir_lowering=False, debug=False, num_devices=NCORES
    )

    v_d = nc.dram_tensor("v", [BL, LV, D], F32, kind="ExternalInput")
    q_d = nc.dram_tensor("q", [BL, LQ, D], F32, kind="ExternalInput")
    wvt_d = nc.dram_tensor("WvT", [D, D], F32, kind="ExternalInput")
    wqt_d = nc.dram_tensor("WqT", [D, D], F32, kind="ExternalInput")
    wh2_d = nc.dram_tensor("Wh2", [D, 2], F32, kind="ExternalInput")
    bv_d = nc.dram_tensor("bv2", [D, 1], F32, kind="ExternalInput")
    bq_d = nc.dram_tensor("bq2", [D, 1], F32, kind="ExternalInput")
    eye_d = nc.dram_tensor("eye", [128, 128], F32, kind="ExternalInput")
    vatt_d = nc.dram_tensor("v_att", [BL, LQ, D], F32, kind="ExternalOutput")
    qatt_d = nc.dram_tensor("q_att", [BL, LV, D], F32, kind="ExternalOutput")

    with tile.TileContext(nc) as tc:
        with (
            tc.tile_pool(name="const", bufs=1) as cpool,
            tc.tile_pool(name="work", bufs=2) as wpool,
            tc.tile_pool(name="spool", bufs=2) as spool,
            tc.tile_pool(name="mm", bufs=3, space="PSUM") as mmpool,
            tc.tile_pool(name="scps", bufs=2, space="PSUM") as scpool,
        ):
            # ---- constants ----
            wvt_sb = cpool.tile([128, NT, D], F32)   # [p, dt, e]
            wqt_sb = cpool.tile([128, NT, D], F32)
            wh2_sb = cpool.tile([128, NT, 2], F32)   # [p, et, j]
            bv_sb = cpool.tile([128, NT], F32)       # [p, et]
            bq_sb = cpool.tile([128, NT], F32)
            eye = cpool.tile([128, 128], F32)
            nc.sync.dma_start(wvt_sb[:], wvt_d[:].rearrange("(t p) e -> p t e", p=128))
            nc.sync.dma_start(wqt_sb[:], wqt_d[:].rearrange("(t p) e -> p t e", p=128))
            nc.sync.dma_start(wh2_sb[:], wh2_d[:].rearrange("(t p) j -> p t j", p=128))
            nc.sync.dma_start(bv_sb[:], bv_d[:].rearrange("(t p) o -> p (t o)", p=128))
            nc.sync.dma_start(bq_sb[:], bq_d[:].rearrange("(t p) o -> p (t o)", p=128))
            nc.sync.dma_start(eye[:], eye_d[:])

            for b in range(BL):
                # ---- load naturals ----
                vn0 = wpool.tile([128, D], F32, tag="vn0")
                vn1 = wpool.tile([LV1, D], F32, tag="vn1")
                qn = wpool.tile([LQ, D], F32, tag="qn")
                nc.sync.dma_start(vn0[:], v_d[b, 0:128, :])
                nc.sync.dma_start(vn1[:], v_d[b, 128:LV, :])
                nc.sync.dma_start(qn[:], q_d[b, :, :])

                # ---- transpose v, q to [d, *] ----
                vT = wpool.tile([128, NT, LV], F32, tag="vT")
                qT = wpool.tile([128, NT, LQ], F32, tag="qT")
                for dt in range(NT):
                    ps = mmpool.tile([128, LV], F32, tag="mm")
                    cols = slice(dt * 128, (dt + 1) * 128)
                    nc.tensor.transpose(ps[:, 0:128], vn0[:, cols], eye[:])
                    nc.tensor.transpose(ps[:, 128:LV], vn1[:, cols], eye[:LV1, :LV1])
                    nc.vector.tensor_copy(vT[:, dt, :], ps[:])
                for dt in range(NT):
                    ps = mmpool.tile([128, LQ], F32, tag="mm")
                    cols = slice(dt * 128, (dt + 1) * 128)
                    nc.tensor.transpose(ps[:], qn[:, cols], eye[:LQ, :LQ])
                    nc.vector.tensor_copy(qT[:, dt, :], ps[:])

                # ---- projections -> v2T [e, v], q2T [e, q] (bias fused in
                #      eviction) ----
                v2T = wpool.tile([128, NT, LV], F32, tag="v2T")
                q2T = wpool.tile([128, NT, LQ], F32, tag="q2T")
                for et in range(NT):
                    cols = slice(et * 128, (et + 1) * 128)
                    ps = mmpool.tile([128, LV], F32, tag="mm")
                    for dt in range(NT):
                        nc.tensor.matmul(
                            ps[:], lhsT=wvt_sb[:, dt, cols], rhs=vT[:, dt, :],
                            start=(dt == 0), stop=(dt == NT - 1),
                        )
                    nc.vector.tensor_scalar_add(v2T[:, et, :], ps[:], bv_sb[:, et : et + 1])
                for et in range(NT):
                    cols = slice(et * 128, (et + 1) * 128)
                    ps = mmpool.tile([128, LQ], F32, tag="mm")
                    for dt in range(NT):
                        nc.tensor.matmul(
                            ps[:], lhsT=wqt_sb[:, dt, cols], rhs=qT[:, dt, :],
                            start=(dt == 0), stop=(dt == NT - 1),
                        )
                    nc.vector.tensor_scalar_add(q2T[:, et, :], ps[:], bq_sb[:, et : et + 1])

                # ---- natural-layout copies for the output matmuls ----
                q2n = wpool.tile([LQ, D], F32, tag="q2n")       # [q, e]
                ps_q2n = mmpool.tile([LQ, D], F32, tag="mm")
                for et in range(NT):
                    cols = slice(et * 128, (et + 1) * 128)
                    nc.tensor.transpose(ps_q2n[:, cols], q2T[:, et, :], eye[:])
                nc.vector.tensor_copy(q2n[:], ps_q2n[:])

                v2n0 = wpool.tile([128, D], F32, tag="v2n0")    # [v(0:128), e]
                v2n1 = wpool.tile([LV1, D], F32, tag="v2n1")    # [v(128:196), e]
                ps_v2n0 = mmpool.tile([128, D], F32, tag="mm")
                ps_v2n1 = mmpool.tile([LV1, D], F32, tag="mm")
                for et in range(NT):
                    cols = slice(et * 128, (et + 1) * 128)
                    nc.tensor.transpose(ps_v2n0[:, cols], v2T[:, et, 0:128], eye[:])
                    nc.tensor.transpose(ps_v2n1[:, cols], v2T[:, et, 128:LV], eye[:])
                nc.vector.tensor_copy(v2n0[:], ps_v2n0[:])
                nc.vector.tensor_copy(v2n1[:], ps_v2n1[:])

                # ---- S = v2T (+) q2T broadcast add, tanh, score matmuls ----
                scT0 = scpool.tile([128, 2 * LQ], F32, tag="sc0")  # [v0, 2q+j]
                scT1 = scpool.tile([LV1, 2 * LQ], F32, tag="sc1")  # [v1, 2q+j]
                for et in range(NT):
                    S = spool.tile([128, QV], F32, tag="S")
                    s3 = S[:].rearrange("p (a b) -> p a b", a=LQ)
                    v2b = v2T[:, et, :].unsqueeze(1).broadcast_to([128, LQ, LV])
                    q2b = q2T[:, et, :].unsqueeze(2).broadcast_to([128, LQ, LV])
                    eng = nc.gpsimd if (b, et) in _GPSIMD_UNITS else nc.vector
                    eng.tensor_tensor(out=s3, in0=v2b, in1=q2b, op=ALU.add)
                    nc.scalar.activation(S[:], S[:], AF.Tanh)
                    first, last = et == 0, et == NT - 1
                    for qi in range(LQ):
                        base = qi * LV
                        nc.tensor.matmul(
                            scT0[:, 2 * qi : 2 * qi + 2],
                            lhsT=S[:, base : base + 128],
                            rhs=wh2_sb[:, et, :],
                            start=(first and qi == 0),
                            stop=(last and qi == LQ - 1),
                            skip_group_check=True,
                        )
                        nc.tensor.matmul(
                            scT1[:, 2 * qi : 2 * qi + 2],
                            lhsT=S[:, base + 128 : base + LV],
                            rhs=wh2_sb[:, et, :],
                            start=(first and qi == 0),
                            stop=(last and qi == LQ - 1),
                            skip_group_check=True,
                        )

                sc0 = wpool.tile([128, 2 * LQ], F32, tag="sc0sb")
                sc1 = wpool.tile([LV1, 2 * LQ], F32, tag="sc1sb")
                nc.vector.tensor_copy(sc0[:], scT0[:])
                nc.vector.tensor_copy(sc1[:], scT1[:])

                # ---- softmax over v (rows sv) ----
                ps_sv = mmpool.tile([LQ, LV], F32, tag="mm")
                nc.tensor.transpose(ps_sv[:, 0:128], sc0[:, 0::2], eye[:])
                nc.tensor.transpose(ps_sv[:, 128:LV], sc1[:, 0::2], eye[:LV1, :LV1])
                sv = wpool.tile([LQ, LV], F32, tag="sv")
                nc.vector.tensor_copy(sv[:], ps_sv[:])
                nmv = wpool.tile([LQ, 1], F32, tag="nmv")
                nc.vector.tensor_reduce(nmv[:], sv[:], axis=mybir.AxisListType.X,
                                        op=ALU.max, negate=True)
                expv = wpool.tile([LQ, LV], F32, tag="expv")
                nc.scalar.activation(expv[:], sv[:], AF.Exp, bias=nmv[:])
                sumv = wpool.tile([LQ, 1], F32, tag="sumv")
                nc.vector.tensor_reduce(sumv[:], expv[:], axis=mybir.AxisListType.X,
                                        op=ALU.add)
                rv = wpool.tile([LQ, 1], F32, tag="rv")
                nc.vector.reciprocal(rv[:], sumv[:])
                ps_avT0 = mmpool.tile([128, LQ], F32, tag="mm")
                ps_avT1 = mmpool.tile([LV1, LQ], F32, tag="mm")
                nc.tensor.transpose(ps_avT0[:], expv[:, 0:128], eye[:LQ, :LQ])
                nc.tensor.transpose(ps_avT1[:], expv[:, 128:LV], eye[:LQ, :LQ])
                avT0 = wpool.tile([128, LQ], F32, tag="avT0")
                avT1 = wpool.tile([LV1, LQ], F32, tag="avT1")
                nc.vector.tensor_copy(avT0[:], ps_avT0[:])
                nc.vector.tensor_copy(avT1[:], ps_avT1[:])

                # ---- softmax over q (columns sq, already v-partitioned) ----
                nmq0 = wpool.tile([128, 1], F32, tag="nmq0")
                nmq1 = wpool.tile([LV1, 1], F32, tag="nmq1")
                nc.vector.tensor_reduce(nmq0[:], sc0[:, 1::2], axis=mybir.AxisListType.X,
                                        op=ALU.max, negate=True)
                nc.vector.tensor_reduce(nmq1[:], sc1[:, 1::2], axis=mybir.AxisListType.X,
                                        op=ALU.max, negate=True)
                expq0 = wpool.tile([128, LQ], F32, tag="expq0")
                expq1 = wpool.tile([LV1, LQ], F32, tag="expq1")
                nc.scalar.activation(expq0[:], sc0[:, 1::2], AF.Exp, bias=nmq0[:])
                nc.scalar.activation(expq1[:], sc1[:, 1::2], AF.Exp, bias=nmq1[:])
                sq0 = wpool.tile([128, 1], F32, tag="sq0")
                sq1 = wpool.tile([LV1, 1], F32, tag="sq1")
                nc.vector.tensor_reduce(sq0[:], expq0[:], axis=mybir.AxisListType.X,
                                        op=ALU.add)
                nc.vector.tensor_reduce(sq1[:], expq1[:], axis=mybir.AxisListType.X,
                                        op=ALU.add)
                rq0 = wpool.tile([128, 1], F32, tag="rq0")
                rq1 = wpool.tile([LV1, 1], F32, tag="rq1")
                nc.vector.reciprocal(rq0[:], sq0[:])
                nc.vector.reciprocal(rq1[:], sq1[:])
                ps_eq = mmpool.tile([LQ, LV], F32, tag="mm")
                nc.tensor.transpose(ps_eq[:, 0:128], expq0[:], eye[:])
                nc.tensor.transpose(ps_eq[:, 128:LV], expq1[:], eye[:LV1, :LV1])
                expq = wpool.tile([LQ, LV], F32, tag="expq")
                nc.vector.tensor_copy(expq[:], ps_eq[:])

                # ---- v_att = (expv @ v2n) * rv ----
                ps_vatt = mmpool.tile([LQ, D], F32, tag="mm")
                nc.tensor.matmul(ps_vatt[:], lhsT=avT0[:], rhs=v2n0[:],
                                 start=True, stop=False)
                nc.tensor.matmul(ps_vatt[:], lhsT=avT1[:], rhs=v2n1[:],
                                 start=False, stop=True)
                vatt = wpool.tile([LQ, D], F32, tag="vatt")
                nc.scalar.activation(vatt[:], ps_vatt[:], AF.Identity, scale=rv[:])
                nc.sync.dma_start(vatt_d[b, :, :], vatt[:])

                # ---- q_att = (expq.T @ q2n) * rq ----
                ps_qatt0 = mmpool.tile([128, D], F32, tag="mm")
                ps_qatt1 = mmpool.tile([LV1, D], F32, tag="mm")
                nc.tensor.matmul(ps_qatt0[:], lhsT=expq[:, 0:128], rhs=q2n[:],
                                 start=True, stop=True)
                nc.tensor.matmul(ps_qatt1[:], lhsT=expq[:, 128:LV], rhs=q2n[:],
                                 start=True, stop=True)
                qatt0 = wpool.tile([128, D], F32, tag="qatt0")
                qatt1 = wpool.tile([LV1, D], F32, tag="qatt1")
                nc.scalar.activation(qatt0[:], ps_qatt0[:], AF.Identity, scale=rq0[:])
                nc.scalar.activation(qatt1[:], ps_qatt1[:], AF.Identity, scale=rq1[:])
                nc.sync.dma_start(qatt_d[b, 0:128, :], qatt0[:])
                nc.sync.dma_start(qatt_d[b, 128:LV, :], qatt1[:])

    nc.compile()
    return nc


def _get_nc():
    global _NC_CACHE
    if _NC_CACHE is None:
        _NC_CACHE = _build_program()
    return _NC_CACHE


_LAST_RESULTS = None  # BassKernelResults of the most recent run (for test.py)


def _install_ntff_hook():
    """Provide antenv.axon_hooks (absent in this image) so trace=True can
    drive NRT profiling through libaxon_pjrt.so. Mirrors the boot-time
    installer in trn_agent_boot/trn_boot.py."""
    import contextlib
    import ctypes
    import sys
    import types

    if "antenv.axon_hooks" in sys.modules:
        return
    so_path = "/opt/axon/libaxon_pjrt.so"
    try:
        lib = ctypes.CDLL(so_path)
    except OSError:
        return
    if not hasattr(lib, "axon_start_nrt_profile"):
        return
    lib.axon_start_nrt_profile.argtypes = [
        ctypes.POINTER(ctypes.c_int64),
        ctypes.c_size_t,
    ]
    lib.axon_start_nrt_profile.restype = ctypes.c_int64
    lib.axon_stop_nrt_profile.argtypes = [ctypes.c_char_p]
    lib.axon_stop_nrt_profile.restype = ctypes.c_int64

    @contextlib.contextmanager
    def _hook(output_dir, device_ids):
        import jax

        jax.devices()
        if device_ids:
            ids = (ctypes.c_int64 * len(device_ids))(*device_ids)
            rc = lib.axon_start_nrt_profile(ids, len(device_ids))
        else:
            rc = lib.axon_start_nrt_profile(None, 0)
        if rc != 0:
            raise RuntimeError(f"axon_start_nrt_profile rc={rc}")
        try:
            yield
        finally:
            n = lib.axon_stop_nrt_profile(str(output_dir).encode())
            print(f"ntff profile: {n} file(s) written to {output_dir}")

    import antenv

    mod = types.ModuleType("antenv.axon_hooks")
    mod._hook = _hook
    mod.get_axon_ntff_profile_hook = lambda: mod._hook

    def _set(h):
        mod._hook = h

    mod.set_axon_ntff_profile_hook = _set
    sys.modules["antenv.axon_hooks"] = mod
    antenv.axon_hooks = mod


def kernel(v, q, Wv, bv, Wq, bq, Whv, bhv, Whq, bhq, _trace=False):
    global _LAST_RESULTS
    v = np.ascontiguousarray(np.asarray(v, dtype=np.float32))
    q = np.ascontiguousarray(np.asarray(q, dtype=np.float32))
    wvt = np.ascontiguousarray(np.asarray(Wv, dtype=np.float32).T)
    wqt = np.ascontiguousarray(np.asarray(Wq, dtype=np.float32).T)
    wh2 = np.ascontiguousarray(
        np.stack([np.asarray(Whv, np.float32)[0], np.asarray(Whq, np.float32)[0]], axis=1)
    )
    bvc = np.ascontiguousarray(np.asarray(bv, np.float32).reshape(D, 1))
    bqc = np.ascontiguousarray(np.asarray(bq, np.float32).reshape(D, 1))
    eye = np.eye(128, dtype=np.float32)
    # bhv/bhq shift scores by a constant -> no effect after softmax.

    if _trace:
        _install_ntff_hook()
    nc = _get_nc()
    in_maps = []
    for c in range(NCORES):
        sl = slice(c * BL, (c + 1) * BL)
        in_maps.append({
            "v": np.ascontiguousarray(v[sl]),
            "q": np.ascontiguousarray(q[sl]),
            "WvT": wvt, "WqT": wqt, "Wh2": wh2,
            "bv2": bvc, "bq2": bqc, "eye": eye,
        })
    res = run_bass_kernel_spmd(nc, in_maps, list(range(NCORES)), trace=_trace)
    _LAST_RESULTS = res
    v_att = np.concatenate([res.results[c]["v_att"] for c in range(NCORES)], axis=0)
    q_att = np.concatenate([res.results[c]["q_att"] for c in range(NCORES)], axis=0)
    return (v_att, q_att)


# revision 15
# speedup vs baseline: 1.2722x; 1.2722x over previous
"""CoAttention kernel for Trainium2 (Bass/Tile), 8-core data-parallel.

Reference computation (per batch b):
    v2 = v @ Wv.T + bv                  [Lv, D]
    q2 = q @ Wq.T + bq                  [Lq, D]
    h  = tanh(v2[None,:,:] + q2[:,None,:])   [Lq, Lv, D]
    sv = h @ Whv ; sq = h @ Whq         [Lq, Lv]   (+bhv/+bhq: shift-invariant
                                                    under softmax -> dropped)
    av = softmax(sv, axis=v) ; aq = softmax(sq, axis=q)
    v_att = av @ v2                     [Lq, D]
    q_att = aq.T @ q2                   [Lv, D]

Sharding: pure data parallel over batch (16 batches / 8 cores = 2 per core).

Per-core dataflow (all shapes fp32):
  - load v [2,196,512], q [2,32,512]; weights pre-transposed on host:
    WvT/WqT [d,e], Wh2 [e,2] = stack(Whv,Whq), bv/bq [512,1], eye [128,128]
  - PE-transpose v,q -> vT,qT (d on partitions)
  - v2T[e,v] / q2T[e,q] via matmul, bias added during PSUM eviction
  - per e-tile (4 of 128): S[e, q*196+v] = v2T[e,v] + q2T[e,q] via ONE
    broadcast tensor_tensor add (DVE or GPSIMD, balanced), then one big
    in-place tanh on ACT ([128, 6272] per instruction)
  - scores, transposed orientation: scT[v, 2q+j] = sum_e S[e,qv] * Wh2[e,j]
    PE matmuls with S slices stationary, accumulated across e-tiles in 2
    PSUM banks (single start/stop per bank, interleaved groups)
  - softmax: sv rows via PE transpose (reduce over free v), sq columns
    directly (reduce over free q); normalization folded into the output
    matmul PSUM eviction as a per-partition ACT scale
  - v_att = exp(sv)@v2 * rv ; q_att = exp(sq).T@q2 * rq ; DMA out
"""

import numpy as np

import concourse.bass as bass
import concourse.mybir as mybir
import concourse.tile as tile
from concourse import bacc
from concourse.bass_utils import run_bass_kernel_spmd

F32 = mybir.dt.float32
AF = mybir.ActivationFunctionType
ALU = mybir.AluOpType

B, LV, LQ, D = 16, 196, 32, 512
NCORES = 8
BL = B // NCORES          # batches per core
NT = D // 128             # 4 partition tiles of the feature dims
LV0, LV1 = 128, LV - 128  # 196 = 128 + 68
QV = LQ * LV              # 6272 = flattened (q, v) free dim

# (batch, half, etile) units whose broadcast-add runs on GPSIMD instead of
# DVE (DVE ~3.4us/unit, GPSIMD ~6.9us/unit; 5/16 on GPSIMD balances both
# under the ~44us ACT tanh floor)
_GPSIMD_UNITS = {(0, 0, 1), (0, 1, 3), (1, 0, 0), (1, 0, 2), (1, 1, 2)}

_NC_CACHE = None


def _build_program():
    nc = bacc.Bacc(
        "TRN2", target_bir_lowering=False, debug=False, num_devices=NCORES
    )

    v_d = nc.dram_tensor("v", [BL, LV, D], F32, kind="ExternalInput")
    q_d = nc.dram_tensor("q", [BL, LQ, D], F32, kind="ExternalInput")
    wvt_d = nc.dram_tensor("WvT", [D, D], F32, kind="ExternalInput")
    wqt_d = nc.dram_tensor("WqT", [D, D], F32, kind="ExternalInput")
    wh2_d = nc.dram_tensor("Wh2", [D, 2], F32, kind="ExternalInput")
    bv_d = nc.dram_tensor("bv2", [D, 1], F32, kind="ExternalInput")
    bq_d = nc.dram_tensor("bq2", [D, 1], F32, kind="ExternalInput")
    eye_d = nc.dram_tensor("eye", [128, 128], F32, kind="ExternalInput")
    vatt_d = nc.dram_tensor("v_att", [BL, LQ, D], F32, kind="ExternalOutput")
    qatt_d = nc.dram_tensor("q_att", [BL, LV, D], F32, kind="ExternalOutput")

    with tile.TileContext(nc) as tc:
        with (
            tc.tile_pool(name="const", bufs=1) as cpool,
            tc.tile_pool(name="work", bufs=2) as wpool,
            tc.tile_pool(name="spool", bufs=1) as spool,
            tc.tile_pool(name="bigpool", bufs=1) as bpool,
            tc.tile_pool(name="mm", bufs=3, space="PSUM") as mmpool,
            tc.tile_pool(name="scps", bufs=3, space="PSUM") as scpool,
        ):
            # ---- constants ----
            wvt_sb = cpool.tile([128, NT, D], F32)   # [p, dt, e]
            wqt_sb = cpool.tile([128, NT, D], F32)
            wh2_sb = cpool.tile([128, NT, 2], F32)   # [p, et, j]
            bv_sb = cpool.tile([128, NT], F32)       # [p, et]
            bq_sb = cpool.tile([128, NT], F32)
            eye = cpool.tile([128, 128], F32)
            nc.sync.dma_start(eye[:], eye_d[:])
            nc.sync.dma_start(wvt_sb[:], wvt_d[:].rearrange("(t p) e -> p t e", p=128))
            nc.sync.dma_start(wqt_sb[:], wqt_d[:].rearrange("(t p) e -> p t e", p=128))
            nc.sync.dma_start(wh2_sb[:], wh2_d[:].rearrange("(t p) j -> p t j", p=128))
            nc.sync.dma_start(bv_sb[:], bv_d[:].rearrange("(t p) o -> p (t o)", p=128))
            nc.sync.dma_start(bq_sb[:], bq_d[:].rearrange("(t p) o -> p (t o)", p=128))

            # HAM warm-up: ~5us of junk PE work as soon as the identity lands,
            # so the clock gate opens (1.2 -> 2.4 GHz) before real matmuls.
            warm = mmpool.tile([128, 128], F32, tag="mm")
            for _ in range(24):
                nc.tensor.transpose(warm[:], eye[:], eye[:])

            for b in range(BL):
                # ---- load naturals ----
                vn0 = wpool.tile([128, D], F32, tag="vn0")
                vn1 = wpool.tile([LV1, D], F32, tag="vn1")
                qn = wpool.tile([LQ, D], F32, tag="qn")
                nc.sync.dma_start(vn0[:], v_d[b, 0:128, :])
                nc.sync.dma_start(vn1[:], v_d[b, 128:LV, :])
                nc.sync.dma_start(qn[:], q_d[b, :, :])

                # ---- transpose v, q to [d, *] ----
                vT = wpool.tile([128, NT, LV], F32, tag="vT")
                qT = wpool.tile([128, NT, LQ], F32, tag="qT")
                for dt in range(NT):
                    ps = mmpool.tile([128, LV], F32, tag="mm")
                    cols = slice(dt * 128, (dt + 1) * 128)
                    nc.tensor.transpose(ps[:, 0:128], vn0[:, cols], eye[:])
                    nc.tensor.transpose(ps[:, 128:LV], vn1[:, cols], eye[:LV1, :LV1])
                    nc.vector.tensor_copy(vT[:, dt, :], ps[:])
                for dt in range(NT):
                    ps = mmpool.tile([128, LQ], F32, tag="mm")
                    cols = slice(dt * 128, (dt + 1) * 128)
                    nc.tensor.transpose(ps[:], qn[:, cols], eye[:LQ, :LQ])
                    nc.vector.tensor_copy(qT[:, dt, :], ps[:])

                # ---- projections -> v2T [e, v], q2T [e, q] (bias fused in
                #      eviction) ----
                v2T = wpool.tile([128, NT, LV], F32, tag="v2T")
                q2T = wpool.tile([128, NT, LQ], F32, tag="q2T")
                for et in range(NT):
                    cols = slice(et * 128, (et + 1) * 128)
                    ps = mmpool.tile([128, LV], F32, tag="mm")
                    for dt in range(NT):
                        nc.tensor.matmul(
                            ps[:], lhsT=wvt_sb[:, dt, cols], rhs=vT[:, dt, :],
                            start=(dt == 0), stop=(dt == NT - 1),
                        )
                    nc.vector.tensor_scalar_add(v2T[:, et, :], ps[:], bv_sb[:, et : et + 1])
                for et in range(NT):
                    cols = slice(et * 128, (et + 1) * 128)
                    ps = mmpool.tile([128, LQ], F32, tag="mm")
                    for dt in range(NT):
                        nc.tensor.matmul(
                            ps[:], lhsT=wqt_sb[:, dt, cols], rhs=qT[:, dt, :],
                            start=(dt == 0), stop=(dt == NT - 1),
                        )
                    nc.vector.tensor_scalar_add(q2T[:, et, :], ps[:], bq_sb[:, et : et + 1])

                # ---- natural-layout copies for the output matmuls ----
                q2n = wpool.tile([LQ, D], F32, tag="q2n")       # [q, e]
                ps_q2n = mmpool.tile([LQ, D], F32, tag="mm")
                for et in range(NT):
                    cols = slice(et * 128, (et + 1) * 128)
                    nc.tensor.transpose(ps_q2n[:, cols], q2T[:, et, :], eye[:])
                nc.vector.tensor_copy(q2n[:], ps_q2n[:])

                v2n0 = wpool.tile([128, D], F32, tag="v2n0")    # [v(0:128), e]
                v2n1 = wpool.tile([LV1, D], F32, tag="v2n1")    # [v(128:196), e]
                ps_v2n0 = mmpool.tile([128, D], F32, tag="mm")
                ps_v2n1 = mmpool.tile([LV1, D], F32, tag="mm")
                for et in range(NT):
                    cols = slice(et * 128, (et + 1) * 128)
                    nc.tensor.transpose(ps_v2n0[:, cols], v2T[:, et, 0:128], eye[:])
                    nc.tensor.transpose(ps_v2n1[:, cols], v2T[:, et, 128:LV], eye[:])
                nc.vector.tensor_copy(v2n0[:], ps_v2n0[:])
                nc.vector.tensor_copy(v2n1[:], ps_v2n1[:])

                # ---- S = v2T (+) q2T broadcast add + tanh, built per
                #      q-half ([128, 3136] tiles, 4 e-tiles resident), then
                #      score matmuls per q-pair: tiny 2-col Wh2 stationary,
                #      [2, 392] PSUM accumulated over e-tiles ----
                QH = LQ // 2  # 16 q's per half
                sc_sb = bpool.tile([2, LQ // 2, 2 * LV], F32, tag="scsb")
                for half in range(2):
                    S_list = []
                    for et in range(NT):
                        S = spool.tile([128, QH * LV], F32, tag=f"S{et}")
                        s3 = S[:].rearrange("p (a b) -> p a b", a=QH)
                        v2b = v2T[:, et, :].unsqueeze(1).broadcast_to(
                            [128, QH, LV])
                        q2b = q2T[:, et, QH * half : QH * (half + 1)].unsqueeze(
                            2).broadcast_to([128, QH, LV])
                        eng = (nc.gpsimd if (b, half, et) in _GPSIMD_UNITS
                               else nc.vector)
                        eng.tensor_tensor(out=s3, in0=v2b, in1=q2b, op=ALU.add)
                        nc.scalar.activation(S[:], S[:], AF.Tanh)
                        S_list.append(S)

                    for pl in range(QH // 2):
                        p = half * (QH // 2) + pl
                        ps_p = scpool.tile([2, 2 * LV], F32, tag="scq")
                        for et in range(NT):
                            nc.tensor.matmul(
                                ps_p[:],
                                lhsT=wh2_sb[:, et, :],
                                rhs=S_list[et][:, 2 * pl * LV : (2 * pl + 2) * LV],
                                start=(et == 0),
                                stop=(et == NT - 1),
                            )
                        if pl % 2 == 0:
                            nc.vector.tensor_copy(sc_sb[:, p, :], ps_p[:])
                        else:
                            nc.scalar.copy(sc_sb[:, p, :], ps_p[:])

                # rearrange [r, p, (q2, v)] -> [q, r, v] rows via 2 DMAs
                svq = wpool.tile([LQ, 2, LV], F32, tag="svq")
                nc.sync.dma_start(svq[:, 0, :], sc_sb[0:1, :, :])
                nc.sync.dma_start(svq[:, 1, :], sc_sb[1:2, :, :])

                # ---- softmax over v (sv rows, already [q, v]) ----
                sv = svq[:, 0, :]
                nmv = wpool.tile([LQ, 1], F32, tag="nmv")
                nc.vector.tensor_reduce(nmv[:], sv, axis=mybir.AxisListType.X,
                                        op=ALU.max, negate=True)
                expv = wpool.tile([LQ, LV], F32, tag="expv")
                nc.scalar.activation(expv[:], sv, AF.Exp, bias=nmv[:])
                sumv = wpool.tile([LQ, 1], F32, tag="sumv")
                nc.vector.tensor_reduce(sumv[:], expv[:], axis=mybir.AxisListType.X,
                                        op=ALU.add)
                rv = wpool.tile([LQ, 1], F32, tag="rv")
                nc.vector.reciprocal(rv[:], sumv[:])
                ps_avT0 = mmpool.tile([128, LQ], F32, tag="mm")
                ps_avT1 = mmpool.tile([LV1, LQ], F32, tag="mm")
                nc.tensor.transpose(ps_avT0[:], expv[:, 0:128], eye[:LQ, :LQ])
                nc.tensor.transpose(ps_avT1[:], expv[:, 128:LV], eye[:LQ, :LQ])
                avT0 = wpool.tile([128, LQ], F32, tag="avT0")
                avT1 = wpool.tile([LV1, LQ], F32, tag="avT1")
                nc.vector.tensor_copy(avT0[:], ps_avT0[:])
                nc.vector.tensor_copy(avT1[:], ps_avT1[:])

                # ---- softmax over q (sq needs [v, q] layout) ----
                sq_rows = svq[:, 1, :]  # [q, v]
                ps_sqT0 = mmpool.tile([128, LQ], F32, tag="mm")
                ps_sqT1 = mmpool.tile([LV1, LQ], F32, tag="mm")
                nc.tensor.transpose(ps_sqT0[:], sq_rows[:, 0:128], eye[:LQ, :LQ])
                nc.tensor.transpose(ps_sqT1[:], sq_rows[:, 128:LV], eye[:LQ, :LQ])
                sqT0 = wpool.tile([128, LQ], F32, tag="sqT0")
                sqT1 = wpool.tile([LV1, LQ], F32, tag="sqT1")
                nc.vector.tensor_copy(sqT0[:], ps_sqT0[:])
                nc.vector.tensor_copy(sqT1[:], ps_sqT1[:])
                nmq0 = wpool.tile([128, 1], F32, tag="nmq0")
                nmq1 = wpool.tile([LV1, 1], F32, tag="nmq1")
                nc.vector.tensor_reduce(nmq0[:], sqT0[:], axis=mybir.AxisListType.X,
                                        op=ALU.max, negate=True)
                nc.vector.tensor_reduce(nmq1[:], sqT1[:], axis=mybir.AxisListType.X,
                                        op=ALU.max, negate=True)
                expq0 = wpool.tile([128, LQ], F32, tag="expq0")
                expq1 = wpool.tile([LV1, LQ], F32, tag="expq1")
                nc.scalar.activation(expq0[:], sqT0[:], AF.Exp, bias=nmq0[:])
                nc.scalar.activation(expq1[:], sqT1[:], AF.Exp, bias=nmq1[:])
                sq0 = wpool.tile([128, 1], F32, tag="sq0")
                sq1 = wpool.tile([LV1, 1], F32, tag="sq1")
                nc.vector.tensor_reduce(sq0[:], expq0[:], axis=mybir.AxisListType.X,
                                        op=ALU.add)
                nc.vector.tensor_reduce(sq1[:], expq1[:], axis=mybir.AxisListType.X,
                                        op=ALU.add)
                rq0 = wpool.tile([128, 1], F32, tag="rq0")
                rq1 = wpool.tile([LV1, 1], F32, tag="rq1")
                nc.vector.reciprocal(rq0[:], sq0[:])
                nc.vector.reciprocal(rq1[:], sq1[:])
                ps_eq = mmpool.tile([LQ, LV], F32, tag="mm")
                nc.tensor.transpose(ps_eq[:, 0:128], expq0[:], eye[:])
                nc.tensor.transpose(ps_eq[:, 128:LV], expq1[:], eye[:LV1, :LV1])
                expq = wpool.tile([LQ, LV], F32, tag="expq")
                nc.vector.tensor_copy(expq[:], ps_eq[:])

                # ---- v_att = (expv @ v2n) * rv ----
                ps_vatt = mmpool.tile([LQ, D], F32, tag="mm")
                nc.tensor.matmul(ps_vatt[:], lhsT=avT0[:], rhs=v2n0[:],
                                 start=True, stop=False)
                nc.tensor.matmul(ps_vatt[:], lhsT=avT1[:], rhs=v2n1[:],
                                 start=False, stop=True)
                vatt = wpool.tile([LQ, D], F32, tag="vatt")
                nc.scalar.activation(vatt[:], ps_vatt[:], AF.Identity, scale=rv[:])
                nc.sync.dma_start(vatt_d[b, :, :], vatt[:])

                # ---- q_att = (expq.T @ q2n) * rq ----
                ps_qatt0 = mmpool.tile([128, D], F32, tag="mm")
                ps_qatt1 = mmpool.tile([LV1, D], F32, tag="mm")
                nc.tensor.matmul(ps_qatt0[:], lhsT=expq[:, 0:128], rhs=q2n[:],
                                 start=True, stop=True)
                nc.tensor.matmul(ps_qatt1[:], lhsT=expq[:, 128:LV], rhs=q2n[:],
                                 start=True, stop=True)
                qatt0 = wpool.tile([128, D], F32, tag="qatt0")
                qatt1 = wpool.tile([LV1, D], F32, tag="qatt1")
                nc.scalar.activation(qatt0[:], ps_qatt0[:], AF.Identity, scale=rq0[:])
                nc.scalar.activation(qatt1[:], ps_qatt1[:], AF.Identity, scale=rq1[:])
                nc.sync.dma_start(qatt_d[b, 0:128, :], qatt0[:])
                nc.sync.dma_start(qatt_d[b, 128:LV, :], qatt1[:])

    nc.compile()
    return nc


def _get_nc():
    global _NC_CACHE
    if _NC_CACHE is None:
        _NC_CACHE = _build_program()
    return _NC_CACHE


_LAST_RESULTS = None  # BassKernelResults of the most recent run (for test.py)


def _install_ntff_hook():
    """Provide antenv.axon_hooks (absent in this image) so trace=True can
    drive NRT profiling through libaxon_pjrt.so. Mirrors the boot-time
    installer in trn_agent_boot/trn_boot.py."""
    import contextlib
    import ctypes
    import sys
    import types

    if "antenv.axon_hooks" in sys.modules:
        return
    so_path = "/opt/axon/libaxon_pjrt.so"
    try:
        lib = ctypes.CDLL(so_path)
    except OSError:
        return
    if not hasattr(lib, "axon_start_nrt_profile"):
        return
    lib.axon_start_nrt_profile.argtypes = [
        ctypes.POINTER(ctypes.c_int64),
        ctypes.c_size_t,
    ]
    lib.axon_start_nrt_profile.restype = ctypes.c_int64
    lib.axon_stop_nrt_profile.argtypes = [ctypes.c_char_p]
    lib.axon_stop_nrt_profile.restype = ctypes.c_int64

    @contextlib.contextmanager
    def _hook(output_dir, device_ids):
        import jax

        jax.devices()
        if device_ids:
            ids = (ctypes.c_int64 * len(device_ids))(*device_ids)
            rc = lib.axon_start_nrt_profile(ids, len(device_ids))
        else:
            rc = lib.axon_start_nrt_profile(None, 0)
        if rc != 0:
            raise RuntimeError(f"axon_start_nrt_profile rc={rc}")
        try:
            yield
        finally:
            n = lib.axon_stop_nrt_profile(str(output_dir).encode())
            print(f"ntff profile: {n} file(s) written to {output_dir}")

    import antenv

    mod = types.ModuleType("antenv.axon_hooks")
    mod._hook = _hook
    mod.get_axon_ntff_profile_hook = lambda: mod._hook

    def _set(h):
        mod._hook = h

    mod.set_axon_ntff_profile_hook = _set
    sys.modules["antenv.axon_hooks"] = mod
    antenv.axon_hooks = mod


def kernel(v, q, Wv, bv, Wq, bq, Whv, bhv, Whq, bhq, _trace=False):
    global _LAST_RESULTS
    v = np.ascontiguousarray(np.asarray(v, dtype=np.float32))
    q = np.ascontiguousarray(np.asarray(q, dtype=np.float32))
    wvt = np.ascontiguousarray(np.asarray(Wv, dtype=np.float32).T)
    wqt = np.ascontiguousarray(np.asarray(Wq, dtype=np.float32).T)
    wh2 = np.ascontiguousarray(
        np.stack([np.asarray(Whv, np.float32)[0], np.asarray(Whq, np.float32)[0]], axis=1)
    )
    bvc = np.ascontiguousarray(np.asarray(bv, np.float32).reshape(D, 1))
    bqc = np.ascontiguousarray(np.asarray(bq, np.float32).reshape(D, 1))
    eye = np.eye(128, dtype=np.float32)
    # bhv/bhq shift scores by a constant -> no effect after softmax.

    if _trace:
        _install_ntff_hook()
    nc = _get_nc()
    in_maps = []
    for c in range(NCORES):
        sl = slice(c * BL, (c + 1) * BL)
        in_maps.append({
            "v": np.ascontiguousarray(v[sl]),
            "q": np.ascontiguousarray(q[sl]),
            "WvT": wvt, "WqT": wqt, "Wh2": wh2,
            "bv2": bvc, "bq2": bqc, "eye": eye,
        })
    res = run_bass_kernel_spmd(nc, in_maps, list(range(NCORES)), trace=_trace)
    _LAST_RESULTS = res
    v_att = np.concatenate([res.results[c]["v_att"] for c in range(NCORES)], axis=0)
    q_att = np.concatenate([res.results[c]["q_att"] for c in range(NCORES)], axis=0)
    return (v_att, q_att)


# revision 21
# speedup vs baseline: 1.3331x; 1.0478x over previous
"""CoAttention kernel for Trainium2 (Bass/Tile), 8-core data-parallel.

Reference computation (per batch b):
    v2 = v @ Wv.T + bv                  [Lv, D]
    q2 = q @ Wq.T + bq                  [Lq, D]
    h  = tanh(v2[None,:,:] + q2[:,None,:])   [Lq, Lv, D]
    sv = h @ Whv ; sq = h @ Whq         [Lq, Lv]   (+bhv/+bhq: shift-invariant
                                                    under softmax -> dropped)
    av = softmax(sv, axis=v) ; aq = softmax(sq, axis=q)
    v_att = av @ v2                     [Lq, D]
    q_att = aq.T @ q2                   [Lv, D]

Sharding: pure data parallel over batch (16 batches / 8 cores = 2 per core).

Per-core dataflow (all shapes fp32):
  - load v [2,196,512], q [2,32,512]; weights pre-transposed on host:
    WvT/WqT [d,e], Wh2 [e,2] = stack(Whv,Whq), bv/bq [512,1], eye [128,128]
  - PE-transpose v,q -> vT,qT (d on partitions)
  - v2T[e,v] / q2T[e,q] via matmul, bias added during PSUM eviction
  - per e-tile (4 of 128): S[e, q*196+v] = v2T[e,v] + q2T[e,q] via ONE
    broadcast tensor_tensor add (DVE or GPSIMD, balanced), then one big
    in-place tanh on ACT ([128, 6272] per instruction)
  - scores, transposed orientation: scT[v, 2q+j] = sum_e S[e,qv] * Wh2[e,j]
    PE matmuls with S slices stationary, accumulated across e-tiles in 2
    PSUM banks (single start/stop per bank, interleaved groups)
  - softmax: sv rows via PE transpose (reduce over free v), sq columns
    directly (reduce over free q); normalization folded into the output
    matmul PSUM eviction as a per-partition ACT scale
  - v_att = exp(sv)@v2 * rv ; q_att = exp(sq).T@q2 * rq ; DMA out
"""

import numpy as np

import concourse.bass as bass
import concourse.mybir as mybir
import concourse.tile as tile
from concourse import bacc
from concourse.bass_utils import run_bass_kernel_spmd

F32 = mybir.dt.float32
AF = mybir.ActivationFunctionType
ALU = mybir.AluOpType

B, LV, LQ, D = 16, 196, 32, 512
NCORES = 8
BL = B // NCORES          # batches per core
NT = D // 128             # 4 partition tiles of the feature dims
LV0, LV1 = 128, LV - 128  # 196 = 128 + 68
QV = LQ * LV              # 6272 = flattened (q, v) free dim

# (batch, half, etile) units whose broadcast-add runs on GPSIMD instead of
# DVE (DVE ~3.4us/unit busy, GPSIMD ~6.9us/unit; 8/16 on GPSIMD balances
# DVE (adds + evictions) against GPSIMD under the ~46us ACT tanh floor)
_GPSIMD_UNITS = {(b, h, et) for b in range(2) for h in range(2)
                 for et in (1, 3)}

_NC_CACHE = None


def _build_program():
    nc = bacc.Bacc(
        "TRN2", target_bir_lowering=False, debug=False, num_devices=NCORES
    )

    v_d = nc.dram_tensor("v", [BL, LV, D], F32, kind="ExternalInput")
    q_d = nc.dram_tensor("q", [BL, LQ, D], F32, kind="ExternalInput")
    wvt_d = nc.dram_tensor("WvT", [D, D], F32, kind="ExternalInput")
    wqt_d = nc.dram_tensor("WqT", [D, D], F32, kind="ExternalInput")
    wh2_d = nc.dram_tensor("Wh2", [D, 2], F32, kind="ExternalInput")
    bv_d = nc.dram_tensor("bv2", [D, 1], F32, kind="ExternalInput")
    bq_d = nc.dram_tensor("bq2", [D, 1], F32, kind="ExternalInput")
    eye_d = nc.dram_tensor("eye", [128, 128], F32, kind="ExternalInput")
    vatt_d = nc.dram_tensor("v_att", [BL, LQ, D], F32, kind="ExternalOutput")
    qatt_d = nc.dram_tensor("q_att", [BL, LV, D], F32, kind="ExternalOutput")

    with tile.TileContext(nc) as tc:
        with (
            tc.tile_pool(name="const", bufs=1) as cpool,
            tc.tile_pool(name="work", bufs=2) as wpool,
            tc.tile_pool(name="spool", bufs=2) as spool,
            tc.tile_pool(name="evpool", bufs=4) as evpool,
            tc.tile_pool(name="mm", bufs=3, space="PSUM") as mmpool,
            tc.tile_pool(name="scps", bufs=3, space="PSUM") as scpool,
        ):
            # ---- constants ----
            wvt_sb = cpool.tile([128, NT, D], F32)   # [p, dt, e]
            wqt_sb = cpool.tile([128, NT, D], F32)
            wh2_sb = cpool.tile([128, NT, 2], F32)   # [p, et, j]
            bv_sb = cpool.tile([128, NT], F32)       # [p, et]
            bq_sb = cpool.tile([128, NT], F32)
            eye = cpool.tile([128, 128], F32)
            nc.sync.dma_start(eye[:], eye_d[:])
            # big weight loads ride SWDGE (gpsimd) queues so the sync HWDGE
            # queues stay free for v/q/score traffic
            wvt_r = wvt_d[:].rearrange("(t p) e -> p t e", p=128)
            wqt_r = wqt_d[:].rearrange("(t p) e -> p t e", p=128)
            for dt in range(NT):
                nc.gpsimd.dma_start(wvt_sb[:, dt, :], wvt_r[:, dt, :])
                nc.gpsimd.dma_start(wqt_sb[:, dt, :], wqt_r[:, dt, :])
            nc.sync.dma_start(wh2_sb[:], wh2_d[:].rearrange("(t p) j -> p t j", p=128))
            nc.sync.dma_start(bv_sb[:], bv_d[:].rearrange("(t p) o -> p (t o)", p=128))
            nc.sync.dma_start(bq_sb[:], bq_d[:].rearrange("(t p) o -> p (t o)", p=128))

            # HAM warm-up: ~5us of junk PE work as soon as the identity lands,
            # so the clock gate opens (1.2 -> 2.4 GHz) before real matmuls.
            warm = mmpool.tile([128, 128], F32, tag="mm")
            for _ in range(24):
                nc.tensor.transpose(warm[:], eye[:], eye[:])

            for b in range(BL):
                # ---- load naturals ----
                vn0 = wpool.tile([128, D], F32, tag="vn0")
                vn1 = wpool.tile([LV1, D], F32, tag="vn1")
                qn = wpool.tile([LQ, D], F32, tag="qn")
                nc.sync.dma_start(vn0[:], v_d[b, 0:128, :])
                nc.sync.dma_start(vn1[:], v_d[b, 128:LV, :])
                nc.sync.dma_start(qn[:], q_d[b, :, :])

                # ---- transpose v, q to [d, *] ----
                vT = wpool.tile([128, NT, LV], F32, tag="vT")
                qT = wpool.tile([128, NT, LQ], F32, tag="qT")
                for dt in range(NT):
                    ps = mmpool.tile([128, LV], F32, tag="mm")
                    cols = slice(dt * 128, (dt + 1) * 128)
                    nc.tensor.transpose(ps[:, 0:128], vn0[:, cols], eye[:])
                    nc.tensor.transpose(ps[:, 128:LV], vn1[:, cols], eye[:LV1, :LV1])
                    nc.vector.tensor_copy(vT[:, dt, :], ps[:])
                for dt in range(NT):
                    ps = mmpool.tile([128, LQ], F32, tag="mm")
                    cols = slice(dt * 128, (dt + 1) * 128)
                    nc.tensor.transpose(ps[:], qn[:, cols], eye[:LQ, :LQ])
                    nc.vector.tensor_copy(qT[:, dt, :], ps[:])

                # ---- projections -> v2T [e, v], q2T [e, q] (bias fused in
                #      eviction) ----
                v2T = wpool.tile([128, NT, LV], F32, tag="v2T")
                q2T = wpool.tile([128, NT, LQ], F32, tag="q2T")
                for et in range(NT):
                    cols = slice(et * 128, (et + 1) * 128)
                    ps = mmpool.tile([128, LV], F32, tag="mm")
                    for dt in range(NT):
                        nc.tensor.matmul(
                            ps[:], lhsT=wvt_sb[:, dt, cols], rhs=vT[:, dt, :],
                            start=(dt == 0), stop=(dt == NT - 1),
                        )
                    nc.vector.tensor_scalar_add(v2T[:, et, :], ps[:], bv_sb[:, et : et + 1])
                for et in range(NT):
                    cols = slice(et * 128, (et + 1) * 128)
                    ps = mmpool.tile([128, LQ], F32, tag="mm")
                    for dt in range(NT):
                        nc.tensor.matmul(
                            ps[:], lhsT=wqt_sb[:, dt, cols], rhs=qT[:, dt, :],
                            start=(dt == 0), stop=(dt == NT - 1),
                        )
                    nc.vector.tensor_scalar_add(q2T[:, et, :], ps[:], bq_sb[:, et : et + 1])

                # ---- natural-layout copies for the output matmuls ----
                q2n = wpool.tile([LQ, D], F32, tag="q2n")       # [q, e]
                ps_q2n = mmpool.tile([LQ, D], F32, tag="mm")
                for et in range(NT):
                    cols = slice(et * 128, (et + 1) * 128)
                    nc.tensor.transpose(ps_q2n[:, cols], q2T[:, et, :], eye[:])
                nc.vector.tensor_copy(q2n[:], ps_q2n[:])

                v2n0 = wpool.tile([128, D], F32, tag="v2n0")    # [v(0:128), e]
                v2n1 = wpool.tile([LV1, D], F32, tag="v2n1")    # [v(128:196), e]
                ps_v2n0 = mmpool.tile([128, D], F32, tag="mm")
                ps_v2n1 = mmpool.tile([LV1, D], F32, tag="mm")
                for et in range(NT):
                    cols = slice(et * 128, (et + 1) * 128)
                    nc.tensor.transpose(ps_v2n0[:, cols], v2T[:, et, 0:128], eye[:])
                    nc.tensor.transpose(ps_v2n1[:, cols], v2T[:, et, 128:LV], eye[:])
                nc.vector.tensor_copy(v2n0[:], ps_v2n0[:])
                nc.vector.tensor_copy(v2n1[:], ps_v2n1[:])

                # ---- S = v2T (+) q2T broadcast add + tanh, built per
                #      q-half ([128, 3136] tiles, 4 e-tiles resident), then
                #      score matmuls per q-pair: tiny 2-col Wh2 stationary,
                #      [2, 392] PSUM accumulated over e-tiles ----
                QH = LQ // 2  # 16 q's per half
                svq = wpool.tile([LQ, 2, LV], F32, tag="svq")
                for half in range(2):
                    S_list = []
                    for et in range(NT):
                        S = spool.tile([128, QH * LV], F32, tag=f"S{et}")
                        s3 = S[:].rearrange("p (a b) -> p a b", a=QH)
                        v2b = v2T[:, et, :].unsqueeze(1).broadcast_to(
                            [128, QH, LV])
                        q2b = q2T[:, et, QH * half : QH * (half + 1)].unsqueeze(
                            2).broadcast_to([128, QH, LV])
                        eng = (nc.gpsimd if (b, half, et) in _GPSIMD_UNITS
                               else nc.vector)
                        eng.tensor_tensor(out=s3, in0=v2b, in1=q2b, op=ALU.add)
                        nc.scalar.activation(S[:], S[:], AF.Tanh)
                        S_list.append(S)

                    for pl in range(QH // 2):
                        p = half * (QH // 2) + pl
                        ps_p = scpool.tile([2, 2 * LV], F32, tag="scq")
                        for et in range(NT):
                            nc.tensor.matmul(
                                ps_p[:],
                                lhsT=wh2_sb[:, et, :],
                                rhs=S_list[et][:, 2 * pl * LV : (2 * pl + 2) * LV],
                                start=(et == 0),
                                stop=(et == NT - 1),
                            )
                        scc = evpool.tile([2, 2 * LV], F32, tag="scc")
                        if pl % 2 == 0:
                            nc.vector.tensor_copy(scc[:], ps_p[:])
                        else:
                            nc.scalar.copy(scc[:], ps_p[:])
                        # [r, (q2, v)] -> svq[2p + q2, r, v]
                        nc.sync.dma_start(svq[2 * p : 2 * p + 2, 0, :],
                                          scc[0:1, :])
                        nc.sync.dma_start(svq[2 * p : 2 * p + 2, 1, :],
                                          scc[1:2, :])

                # ---- softmax over v (sv rows, already [q, v]) ----
                sv = svq[:, 0, :]
                nmv = wpool.tile([LQ, 1], F32, tag="nmv")
                nc.vector.tensor_reduce(nmv[:], sv, axis=mybir.AxisListType.X,
                                        op=ALU.max, negate=True)
                expv = wpool.tile([LQ, LV], F32, tag="expv")
                nc.scalar.activation(expv[:], sv, AF.Exp, bias=nmv[:])
                sumv = wpool.tile([LQ, 1], F32, tag="sumv")
                nc.vector.tensor_reduce(sumv[:], expv[:], axis=mybir.AxisListType.X,
                                        op=ALU.add)
                rv = wpool.tile([LQ, 1], F32, tag="rv")
                nc.vector.reciprocal(rv[:], sumv[:])
                ps_avT0 = mmpool.tile([128, LQ], F32, tag="mm")
                ps_avT1 = mmpool.tile([LV1, LQ], F32, tag="mm")
                nc.tensor.transpose(ps_avT0[:], expv[:, 0:128], eye[:LQ, :LQ])
                nc.tensor.transpose(ps_avT1[:], expv[:, 128:LV], eye[:LQ, :LQ])
                avT0 = wpool.tile([128, LQ], F32, tag="avT0")
                avT1 = wpool.tile([LV1, LQ], F32, tag="avT1")
                nc.vector.tensor_copy(avT0[:], ps_avT0[:])
                nc.vector.tensor_copy(avT1[:], ps_avT1[:])

                # ---- softmax over q (sq needs [v, q] layout) ----
                sq_rows = svq[:, 1, :]  # [q, v]
                ps_sqT0 = mmpool.tile([128, LQ], F32, tag="mm")
                ps_sqT1 = mmpool.tile([LV1, LQ], F32, tag="mm")
                nc.tensor.transpose(ps_sqT0[:], sq_rows[:, 0:128], eye[:LQ, :LQ])
                nc.tensor.transpose(ps_sqT1[:], sq_rows[:, 128:LV], eye[:LQ, :LQ])
                sqT0 = wpool.tile([128, LQ], F32, tag="sqT0")
                sqT1 = wpool.tile([LV1, LQ], F32, tag="sqT1")
                nc.vector.tensor_copy(sqT0[:], ps_sqT0[:])
                nc.vector.tensor_copy(sqT1[:], ps_sqT1[:])
                nmq0 = wpool.tile([128, 1], F32, tag="nmq0")
                nmq1 = wpool.tile([LV1, 1], F32, tag="nmq1")
                nc.vector.tensor_reduce(nmq0[:], sqT0[:], axis=mybir.AxisListType.X,
                                        op=ALU.max, negate=True)
                nc.vector.tensor_reduce(nmq1[:], sqT1[:], axis=mybir.AxisListType.X,
                                        op=ALU.max, negate=True)
                expq0 = wpool.tile([128, LQ], F32, tag="expq0")
                expq1 = wpool.tile([LV1, LQ], F32, tag="expq1")
                nc.scalar.activation(expq0[:], sqT0[:], AF.Exp, bias=nmq0[:])
                nc.scalar.activation(expq1[:], sqT1[:], AF.Exp, bias=nmq1[:])
                sq0 = wpool.tile([128, 1], F32, tag="sq0")
                sq1 = wpool.tile([LV1, 1], F32, tag="sq1")
                nc.vector.tensor_reduce(sq0[:], expq0[:], axis=mybir.AxisListType.X,
                                        op=ALU.add)
                nc.vector.tensor_reduce(sq1[:], expq1[:], axis=mybir.AxisListType.X,
                                        op=ALU.add)
                rq0 = wpool.tile([128, 1], F32, tag="rq0")
                rq1 = wpool.tile([LV1, 1], F32, tag="rq1")
                nc.vector.reciprocal(rq0[:], sq0[:])
                nc.vector.reciprocal(rq1[:], sq1[:])
                ps_eq = mmpool.tile([LQ, LV], F32, tag="mm")
                nc.tensor.transpose(ps_eq[:, 0:128], expq0[:], eye[:])
                nc.tensor.transpose(ps_eq[:, 128:LV], expq1[:], eye[:LV1, :LV1])
                expq = wpool.tile([LQ, LV], F32, tag="expq")
                nc.vector.tensor_copy(expq[:], ps_eq[:])

                # ---- v_att = (expv @ v2n) * rv ----
                ps_vatt = mmpool.tile([LQ, D], F32, tag="mm")
                nc.tensor.matmul(ps_vatt[:], lhsT=avT0[:], rhs=v2n0[:],
                                 start=True, stop=False)
                nc.tensor.matmul(ps_vatt[:], lhsT=avT1[:], rhs=v2n1[:],
                                 start=False, stop=True)
                vatt = wpool.tile([LQ, D], F32, tag="vatt")
                nc.scalar.activation(vatt[:], ps_vatt[:], AF.Identity, scale=rv[:])
                nc.sync.dma_start(vatt_d[b, :, :], vatt[:])

                # ---- q_att = (expq.T @ q2n) * rq ----
                ps_qatt0 = mmpool.tile([128, D], F32, tag="mm")
                ps_qatt1 = mmpool.tile([LV1, D], F32, tag="mm")
                nc.tensor.matmul(ps_qatt0[:], lhsT=expq[:, 0:128], rhs=q2n[:],
                                 start=True, stop=True)
                nc.tensor.matmul(ps_qatt1[:], lhsT=expq[:, 128:LV], rhs=q2n[:],
                                 start=True, stop=True)
                qatt0 = wpool.tile([128, D], F32, tag="qatt0")
                qatt1 = wpool.tile([LV1, D], F32, tag="qatt1")
                nc.scalar.activation(qatt0[:], ps_qatt0[:], AF.Identity, scale=rq0[:])
                nc.scalar.activation(qatt1[:], ps_qatt1[:], AF.Identity, scale=rq1[:])
                nc.sync.dma_start(qatt_d[b, 0:128, :], qatt0[:])
                nc.sync.dma_start(qatt_d[b, 128:LV, :], qatt1[:])

    nc.compile()
    return nc


def _get_nc():
    global _NC_CACHE
    if _NC_CACHE is None:
        _NC_CACHE = _build_program()
    return _NC_CACHE


_LAST_RESULTS = None  # BassKernelResults of the most recent run (for test.py)


def _install_ntff_hook():
    """Provide antenv.axon_hooks (absent in this image) so trace=True can
    drive NRT profiling through libaxon_pjrt.so. Mirrors the boot-time
    installer in trn_agent_boot/trn_boot.py."""
    import contextlib
    import ctypes
    import sys
    import types

    if "antenv.axon_hooks" in sys.modules:
        return
    so_path = "/opt/axon/libaxon_pjrt.so"
    try:
        lib = ctypes.CDLL(so_path)
    except OSError:
        return
    if not hasattr(lib, "axon_start_nrt_profile"):
        return
    lib.axon_start_nrt_profile.argtypes = [
        ctypes.POINTER(ctypes.c_int64),
        ctypes.c_size_t,
    ]
    lib.axon_start_nrt_profile.restype = ctypes.c_int64
    lib.axon_stop_nrt_profile.argtypes = [ctypes.c_char_p]
    lib.axon_stop_nrt_profile.restype = ctypes.c_int64

    @contextlib.contextmanager
    def _hook(output_dir, device_ids):
        import jax

        jax.devices()
        if device_ids:
            ids = (ctypes.c_int64 * len(device_ids))(*device_ids)
            rc = lib.axon_start_nrt_profile(ids, len(device_ids))
        else:
            rc = lib.axon_start_nrt_profile(None, 0)
        if rc != 0:
            raise RuntimeError(f"axon_start_nrt_profile rc={rc}")
        try:
            yield
        finally:
            n = lib.axon_stop_nrt_profile(str(output_dir).encode())
            print(f"ntff profile: {n} file(s) written to {output_dir}")

    import antenv

    mod = types.ModuleType("antenv.axon_hooks")
    mod._hook = _hook
    mod.get_axon_ntff_profile_hook = lambda: mod._hook

    def _set(h):
        mod._hook = h

    mod.set_axon_ntff_profile_hook = _set
    sys.modules["antenv.axon_hooks"] = mod
    antenv.axon_hooks = mod


def kernel(v, q, Wv, bv, Wq, bq, Whv, bhv, Whq, bhq, _trace=False):
    global _LAST_RESULTS
    v = np.ascontiguousarray(np.asarray(v, dtype=np.float32))
    q = np.ascontiguousarray(np.asarray(q, dtype=np.float32))
    wvt = np.ascontiguousarray(np.asarray(Wv, dtype=np.float32).T)
    wqt = np.ascontiguousarray(np.asarray(Wq, dtype=np.float32).T)
    wh2 = np.ascontiguousarray(
        np.stack([np.asarray(Whv, np.float32)[0], np.asarray(Whq, np.float32)[0]], axis=1)
    )
    bvc = np.ascontiguousarray(np.asarray(bv, np.float32).reshape(D, 1))
    bqc = np.ascontiguousarray(np.asarray(bq, np.float32).reshape(D, 1))
    eye = np.eye(128, dtype=np.float32)
    # bhv/bhq shift scores by a constant -> no effect after softmax.

    if _trace:
        _install_ntff_hook()
    nc = _get_nc()
    in_maps = []
    for c in range(NCORES):
        sl = slice(c * BL, (c + 1) * BL)
        in_maps.append({
            "v": np.ascontiguousarray(v[sl]),
            "q": np.ascontiguousarray(q[sl]),
            "WvT": wvt, "WqT": wqt, "Wh2": wh2,
            "bv2": bvc, "bq2": bqc, "eye": eye,
        })
    res = run_bass_kernel_spmd(nc, in_maps, list(range(NCORES)), trace=_trace)
    _LAST_RESULTS = res
    v_att = np.concatenate([res.results[c]["v_att"] for c in range(NCORES)], axis=0)
    q_att = np.concatenate([res.results[c]["q_att"] for c in range(NCORES)], axis=0)
    return (v_att, q_att)
